# revision 13
# baseline (speedup 1.0000x reference)
"""Weighted Chamfer-MSE kernel for Trainium2 (8 NeuronCores, Bass/Tile).

Reference computes, per batch element:
    D[p, q]  = sum_c w[c]^2 * (t[p, c] - y[c, q])^2        (p=2048, q=4096)
    out      = mean_{b,p} min_q D + mean_{b,q} min_p D

Strategy (data-parallel over batch, 1 element per core):
  * Host packs the distance computation into ONE matmul with augmented
    contraction dim K=5:
        D_neg[p, q] = sum_k a[k, p] * b[k, q]
        a = [2*w2_c*t_pc (c=0..2), -wt2_p, -1],  b = [y_cq (c=0..2), 1, wy2_q]
    so D_neg = -D, and both min-reductions become max-reductions.
  * On device, PE produces D_neg in [128, 2048] PSUM tiles; a single fused
    vector.tensor_tensor_reduce per tile maintains
       runq   = elementwise max over p-tiles   (-> min over p, per q)
       rowmax = per-row max                    (-> min over q, per p)
    i.e. the whole 2048x4096 matrix crosses the DVE exactly once.
  * Epilogue: gpsimd partition_all_reduce(max) over runq partitions, small
    DVE add-reductions -> two partial sums per core; host combines.
"""

import os
import numpy as np
from contextlib import ExitStack

from concourse import bacc, bass, tile, mybir
from concourse.bass_isa import ReduceOp
from concourse.bass_utils import run_bass_kernel_spmd
from concourse.dve_spec import Spec, Src0, Src1, MaxNeg, maxx, lower
from concourse.dve_uop import AluInp, DveOpSpec
from concourse.dve_ops import (
    DveOp, OPS, has_src1, CUSTOM_DVE_SPECS,
    _SUB_OPCODE_FOR_NAME, _CUSTOM_DVE_ROW_BASE, _COMPILE_CACHE,
    get_dve_sub_opcode,
)

_B, _C, _H, _W = 8, 3, 64, 64
_P = 2048
_Q = _H * _W  # 4096
_K = 5
_NCORES = 8
_F32 = mybir.dt.float32
_NEG = -3.0e38

# Stashed BassKernelResults from the most recent kernel() call (for test.py).
LAST_RESULTS = None


class _HandEditedMaxMaxOp(DveOp):
    """Custom fused DVE op (uops ship inside the NEFF, no firmware dep):
        out[p,k]     = max(in0[p,k], in1[p,k])   -- elementwise accumulate
        accum_out[p] = max_k in0[p,k]            -- row reduce of in0 ONLY
    The native TENSOR_TENSOR_REDUCE opcode is not implemented by the stock
    DVE firmware on this runtime, and the Spec-level `accum=` folds the body
    output (which would contaminate the row max with in1's history), so the
    steady-state uop is hand-edited: the accum ALU's operand B is repointed
    from the body output (PREV_ALU_OUT) to the raw Src0 delay lane."""

    def compile(self, ver):
        key = (self.name, ver)
        if (r := _COMPILE_CACHE.get(key)) is not None:
            return r
        uops = lower(self.spec, ver=ver)
        assert len(uops) == 2
        uops[1].datapath_config[1].alu_src1 = AluInp.PREV_DELAY_0
        r = DveOpSpec(
            name=self.name,
            opcode=get_dve_sub_opcode(self.name),
            uops=uops,
            rd1_en=has_src1(self.spec),
        )
        _COMPILE_CACHE[key] = r
        return r


def _register_chamfer_op():
    name = "CHAMFER_MAX_SRC0MAX"
    if name in _SUB_OPCODE_FOR_NAME:
        return next(op for op in OPS if op.name == name)
    spec = Spec(
        body=maxx(Src0, Src1),
        accum=maxx,
        accum_init=MaxNeg,
        reference=lambda in0, in1, c0, c1, c2: (
            np.maximum(in0, in1),
            in0.max(axis=-1, keepdims=True),
        ),
    )
    _SUB_OPCODE_FOR_NAME[name] = _CUSTOM_DVE_ROW_BASE + len(OPS)
    op = _HandEditedMaxMaxOp(name, spec, subdim=False, uops_sha={})
    OPS.append(op)
    CUSTOM_DVE_SPECS[name] = spec
    return op


_CHAMFER_OP = _register_chamfer_op()


def _build_nc():
    nc = bacc.Bacc("TRN2", target_bir_lowering=False, debug=False)
    # a and b packed in one DRAM tensor / one DMA so the first (fused-LDW)
    # matmul needs only ONE sync wait -- the S3_LW struct can't hold two.
    ab_dram = nc.dram_tensor("ab", [_K, _P + _Q], _F32, kind="ExternalInput").ap()
    out_dram = nc.dram_tensor("out", [1, 2], _F32, kind="ExternalOutput").ap()

    n_ptile = _P // 128          # 16 stationary tiles
    fd = 2048                    # free-dim per PSUM supertile (4 banks)
    n_qsuper = _Q // fd          # 2
    n_mm = fd // 512             # 4 matmuls per supertile

    with ExitStack() as ctx:
        tc = ctx.enter_context(tile.TileContext(nc))
        sbuf = ctx.enter_context(tc.tile_pool(name="sbuf", bufs=1))
        psum = ctx.enter_context(tc.tile_pool(name="psum", bufs=2, space="PSUM"))

        ab_sb = sbuf.tile([_K, _P + _Q], _F32)
        nc.sync.dma_start(ab_sb[:], ab_dram[:])

        runq = sbuf.tile([128, _Q], _F32)
        nc.gpsimd.memset(runq[:], _NEG)
        rowmax = sbuf.tile([128, n_ptile * n_qsuper], _F32)

        for pi in range(n_ptile):
            lhsT = ab_sb[:, pi * 128:(pi + 1) * 128]
            for qj in range(n_qsuper):
                pt = psum.tile([128, fd], _F32, tag="pt")
                for qk in range(n_mm):
                    q0 = _P + qj * fd + qk * 512
                    nc.tensor.matmul(
                        pt[:, qk * 512:(qk + 1) * 512],
                        lhsT,
                        ab_sb[:, q0:q0 + 512],
                        start=True,
                        stop=True,
                    )
                col = pi * n_qsuper + qj
                qs = runq[:, qj * fd:(qj + 1) * fd]
                nc.vector._custom_dve(
                    _CHAMFER_OP,
                    out=qs,
                    in0=pt[:],
                    in1=qs,
                    accum_out=rowmax[:, col:col + 1],
                )

        # sum_p (max_q D_neg): max over the n_qsuper columns per row, sum
        # over rows/partitions.
        rm2 = sbuf.tile([128, n_ptile], _F32)
        nc.vector.tensor_reduce(
            rm2[:],
            rowmax[:].rearrange("p (a b) -> p a b", b=n_qsuper),
            axis=mybir.AxisListType.X,
            op=mybir.AluOpType.max,
        )
        rsum = sbuf.tile([128, 1], _F32)
        nc.vector.tensor_reduce(
            rsum[:], rm2[:], axis=mybir.AxisListType.X, op=mybir.AluOpType.add
        )
        nc.gpsimd.partition_all_reduce(rsum[:], rsum[:], 128, ReduceOp.add)

        # sum_q (max_p D_neg): partition-max of runq, then free-dim sum.
        nc.gpsimd.partition_all_reduce(runq[:], runq[:], 128, ReduceOp.max)
        qsum = sbuf.tile([128, 1], _F32)
        nc.vector.tensor_reduce(
            qsum[:], runq[:], axis=mybir.AxisListType.X, op=mybir.AluOpType.add
        )

        nc.sync.dma_start(out_dram[0:1, 0:1], rsum[0:1, 0:1])
        nc.sync.dma_start(out_dram[0:1, 1:2], qsum[0:1, 0:1])
    nc.compile()
    return nc


def _pack_inputs(y, t, weights):
    """Build per-core augmented factor matrices (numpy, O((p+q)*c) per core)."""
    w2 = (weights * weights).astype(np.float32)
    in_maps = []
    for i in range(_NCORES):
        yq = y[i].reshape(_C, _Q)
        ti = t[i]
        a = np.empty((_K, _P), np.float32)
        a[0:_C] = (2.0 * w2)[:, None] * ti.T
        a[_C] = -(w2[None, :] * ti * ti).sum(axis=1)
        a[_C + 1] = -1.0
        ab = np.empty((_K, _P + _Q), np.float32)
        ab[:, :_P] = a
        ab[0:_C, _P:] = yq
        ab[_C, _P:] = 1.0
        ab[_C + 1, _P:] = (w2[:, None] * yq * yq).sum(axis=0)
        in_maps.append({"ab": ab})
    return in_maps


def kernel(y, t, weights):
    global LAST_RESULTS
    y = np.asarray(y, dtype=np.float32)
    t = np.asarray(t, dtype=np.float32)
    weights = np.asarray(weights, dtype=np.float32)
    assert y.shape == (_B, _C, _H, _W) and t.shape == (_B, _P, _C)

    in_maps = _pack_inputs(y, t, weights)
    nc = _build_nc()
    trace = bool(os.environ.get("BASS_CHAMFER_TRACE"))
    res = run_bass_kernel_spmd(
        nc, in_maps, core_ids=list(range(_NCORES)), trace=trace
    )
    LAST_RESULTS = res

    bp_neg = sum(float(r["out"][0, 0]) for r in res.results)
    bq_neg = sum(float(r["out"][0, 1]) for r in res.results)
    total = -(bp_neg / (_B * _P) + bq_neg / (_B * _Q))
    return np.float32(total)


# revision 14
# speedup vs baseline: 8.5274x; 8.5274x over previous
"""Weighted Chamfer-MSE kernel for Trainium2 (8 NeuronCores, Bass/Tile).

Reference computes, per batch element:
    D[p, q]  = sum_c w[c]^2 * (t[p, c] - y[c, q])^2        (p=2048, q=4096)
    out      = mean_{b,p} min_q D + mean_{b,q} min_p D

Strategy (data-parallel over batch, 1 element per core):
  * Host packs the distance computation into ONE matmul with augmented
    contraction dim K=5:
        D_neg[p, q] = sum_k a[k, p] * b[k, q]
        a = [2*w2_c*t_pc (c=0..2), -wt2_p, -1],  b = [y_cq (c=0..2), 1, wy2_q]
    so D_neg = -D, and both min-reductions become max-reductions.
  * On device, PE produces D_neg in [128, 2048] PSUM tiles; a single fused
    vector.tensor_tensor_reduce per tile maintains
       runq   = elementwise max over p-tiles   (-> min over p, per q)
       rowmax = per-row max                    (-> min over q, per p)
    i.e. the whole 2048x4096 matrix crosses the DVE exactly once.
  * Epilogue: gpsimd partition_all_reduce(max) over runq partitions, small
    DVE add-reductions -> two partial sums per core; host combines.
"""

import os
import numpy as np
from contextlib import ExitStack

from concourse import bacc, bass, tile, mybir
from concourse.bass_isa import ReduceOp
from concourse.bass_utils import run_bass_kernel_spmd
from concourse.dve_spec import Spec, Src0, Src1, MaxNeg, maxx, lower
from concourse.dve_uop import AluInp, DveOpSpec
from concourse.dve_ops import (
    DveOp, OPS, has_src1, CUSTOM_DVE_SPECS,
    _SUB_OPCODE_FOR_NAME, _CUSTOM_DVE_ROW_BASE, _COMPILE_CACHE,
    get_dve_sub_opcode,
)

_B, _C, _H, _W = 8, 3, 64, 64
_P = 2048
_Q = _H * _W  # 4096
_K = 5
_NCORES = 8
_F32 = mybir.dt.float32
_NEG = -3.0e38

# Stashed BassKernelResults from the most recent kernel() call (for test.py).
LAST_RESULTS = None


class _HandEditedMaxMaxOp(DveOp):
    """Custom fused DVE op (uops ship inside the NEFF, no firmware dep):
        out[p,k]     = max(in0[p,k], in1[p,k])   -- elementwise accumulate
        accum_out[p] = max_k in0[p,k]            -- row reduce of in0 ONLY
    The native TENSOR_TENSOR_REDUCE opcode is not implemented by the stock
    DVE firmware on this runtime, and the Spec-level `accum=` folds the body
    output (which would contaminate the row max with in1's history), so the
    steady-state uop is hand-edited: the accum ALU's operand B is repointed
    from the body output (PREV_ALU_OUT) to the raw Src0 delay lane."""

    def compile(self, ver):
        key = (self.name, ver)
        if (r := _COMPILE_CACHE.get(key)) is not None:
            return r
        uops = lower(self.spec, ver=ver)
        assert len(uops) == 2
        uops[1].datapath_config[1].alu_src1 = AluInp.PREV_DELAY_0
        r = DveOpSpec(
            name=self.name,
            opcode=get_dve_sub_opcode(self.name),
            uops=uops,
            rd1_en=has_src1(self.spec),
        )
        _COMPILE_CACHE[key] = r
        return r


def _register_chamfer_op():
    name = "CHAMFER_MAX_SRC0MAX"
    if name in _SUB_OPCODE_FOR_NAME:
        return next(op for op in OPS if op.name == name)
    spec = Spec(
        body=maxx(Src0, Src1),
        accum=maxx,
        accum_init=MaxNeg,
        reference=lambda in0, in1, c0, c1, c2: (
            np.maximum(in0, in1),
            in0.max(axis=-1, keepdims=True),
        ),
    )
    _SUB_OPCODE_FOR_NAME[name] = _CUSTOM_DVE_ROW_BASE + len(OPS)
    op = _HandEditedMaxMaxOp(name, spec, subdim=False, uops_sha={})
    OPS.append(op)
    CUSTOM_DVE_SPECS[name] = spec
    return op


_CHAMFER_OP = _register_chamfer_op()


def _build_nc():
    nc = bacc.Bacc("TRN2", target_bir_lowering=False, debug=False)
    # a and b packed in one DRAM tensor; split into two DMAs so the first
    # half of the compute can start before the second half's data lands.
    # (The fused-LDW fp32 matmul struct can hold only ONE sync wait, so each
    # matmul must depend on at most one DMA.)
    ab_dram = nc.dram_tensor("ab", [_K, _P + _Q], _F32, kind="ExternalInput").ap()
    out_dram = nc.dram_tensor("out", [1, 2], _F32, kind="ExternalOutput").ap()

    n_ptile = _P // 128          # 16 stationary tiles
    fd = 2048                    # free-dim per PSUM supertile (4 banks)
    n_qsuper = _Q // fd          # 2
    n_mm = fd // 512             # 4 matmuls per supertile

    with ExitStack() as ctx:
        tc = ctx.enter_context(tile.TileContext(nc))
        sbuf = ctx.enter_context(tc.tile_pool(name="sbuf", bufs=1))
        psum = ctx.enter_context(tc.tile_pool(name="psum", bufs=2, space="PSUM"))

        ab_sb = sbuf.tile([_K, _P + _Q], _F32)
        # DMA 1: a (cols 0:P) + b first half; DMA 2: b second half.
        nc.sync.dma_start(ab_sb[:, 0:_P + fd], ab_dram[:, 0:_P + fd])
        nc.sync.dma_start(ab_sb[:, _P + fd:], ab_dram[:, _P + fd:])

        runq = sbuf.tile([128, _Q], _F32)
        nc.gpsimd.memset(runq[:], _NEG)
        rowmax = sbuf.tile([128, n_ptile * n_qsuper], _F32)
        qsum_h = sbuf.tile([128, n_qsuper], _F32)

        # qj OUTER: each runq half is final at the half-way point, so its
        # partition-reduce + row-sum epilogue hides under the other half's
        # main loop.
        for qj in range(n_qsuper):
            for pi in range(n_ptile):
                lhsT = ab_sb[:, pi * 128:(pi + 1) * 128]
                pt = psum.tile([128, fd], _F32, tag="pt")
                for qk in range(n_mm):
                    q0 = _P + qj * fd + qk * 512
                    nc.tensor.matmul(
                        pt[:, qk * 512:(qk + 1) * 512],
                        lhsT,
                        ab_sb[:, q0:q0 + 512],
                        start=True,
                        stop=True,
                    )
                col = pi * n_qsuper + qj
                qs = runq[:, qj * fd:(qj + 1) * fd]
                nc.vector._custom_dve(
                    _CHAMFER_OP,
                    out=qs,
                    in0=pt[:],
                    in1=qs,
                    accum_out=rowmax[:, col:col + 1],
                )
            # per-half epilogue: max over partitions (gpsimd), then row-sum
            # on the otherwise-idle ACT engine (accum_out of an in-place
            # copy); every partition holds the same row, so row 0's sum is
            # sum_q max_p for this half.
            qs = runq[:, qj * fd:(qj + 1) * fd]
            nc.gpsimd.partition_all_reduce(qs, qs, 128, ReduceOp.max)
            nc.scalar.activation(
                qs, qs, mybir.ActivationFunctionType.Copy,
                accum_out=qsum_h[:, qj:qj + 1],
            )

        # sum_p (max_q D_neg): max over the n_qsuper columns per row, sum
        # over rows/partitions.
        rm2 = sbuf.tile([128, n_ptile], _F32)
        nc.vector.tensor_reduce(
            rm2[:],
            rowmax[:].rearrange("p (a b) -> p a b", b=n_qsuper),
            axis=mybir.AxisListType.X,
            op=mybir.AluOpType.max,
        )
        rsum = sbuf.tile([128, 1], _F32)
        nc.vector.tensor_reduce(
            rsum[:], rm2[:], axis=mybir.AxisListType.X, op=mybir.AluOpType.add
        )
        nc.gpsimd.partition_all_reduce(rsum[:], rsum[:], 128, ReduceOp.add)

        qsum = sbuf.tile([128, 1], _F32)
        nc.vector.tensor_reduce(
            qsum[:], qsum_h[:], axis=mybir.AxisListType.X, op=mybir.AluOpType.add
        )

        nc.sync.dma_start(out_dram[0:1, 0:1], rsum[0:1, 0:1])
        nc.sync.dma_start(out_dram[0:1, 1:2], qsum[0:1, 0:1])
    nc.compile()
    return nc


def _pack_inputs(y, t, weights):
    """Build per-core augmented factor matrices (numpy, O((p+q)*c) per core)."""
    w2 = (weights * weights).astype(np.float32)
    in_maps = []
    for i in range(_NCORES):
        yq = y[i].reshape(_C, _Q)
        ti = t[i]
        a = np.empty((_K, _P), np.float32)
        a[0:_C] = (2.0 * w2)[:, None] * ti.T
        a[_C] = -(w2[None, :] * ti * ti).sum(axis=1)
        a[_C + 1] = -1.0
        ab = np.empty((_K, _P + _Q), np.float32)
        ab[:, :_P] = a
        ab[0:_C, _P:] = yq
        ab[_C, _P:] = 1.0
        ab[_C + 1, _P:] = (w2[:, None] * yq * yq).sum(axis=0)
        in_maps.append({"ab": ab})
    return in_maps


def kernel(y, t, weights):
    global LAST_RESULTS
    y = np.asarray(y, dtype=np.float32)
    t = np.asarray(t, dtype=np.float32)
    weights = np.asarray(weights, dtype=np.float32)
    assert y.shape == (_B, _C, _H, _W) and t.shape == (_B, _P, _C)

    in_maps = _pack_inputs(y, t, weights)
    nc = _build_nc()
    trace = bool(os.environ.get("BASS_CHAMFER_TRACE"))
    res = run_bass_kernel_spmd(
        nc, in_maps, core_ids=list(range(_NCORES)), trace=trace
    )
    LAST_RESULTS = res

    bp_neg = sum(float(r["out"][0, 0]) for r in res.results)
    bq_neg = sum(float(r["out"][0, 1]) for r in res.results)
    total = -(bp_neg / (_B * _P) + bq_neg / (_B * _Q))
    return np.float32(total)


# revision 16
# speedup vs baseline: 25.1332x; 2.9473x over previous
"""Weighted Chamfer-MSE kernel for Trainium2 (8 NeuronCores, Bass/Tile).

Reference computes, per batch element:
    D[p, q]  = sum_c w[c]^2 * (t[p, c] - y[c, q])^2        (p=2048, q=4096)
    out      = mean_{b,p} min_q D + mean_{b,q} min_p D

Strategy (data-parallel over batch, 1 element per core):
  * Host packs the distance computation into ONE matmul with augmented
    contraction dim K=5:
        D_neg[p, q] = sum_k a[k, p] * b[k, q]
        a = [2*w2_c*t_pc (c=0..2), -wt2_p, -1],  b = [y_cq (c=0..2), 1, wy2_q]
    so D_neg = -D, and both min-reductions become max-reductions.
  * On device, PE produces D_neg in [128, 2048] PSUM tiles; a single fused
    vector.tensor_tensor_reduce per tile maintains
       runq   = elementwise max over p-tiles   (-> min over p, per q)
       rowmax = per-row max                    (-> min over q, per p)
    i.e. the whole 2048x4096 matrix crosses the DVE exactly once.
  * Epilogue: gpsimd partition_all_reduce(max) over runq partitions, small
    DVE add-reductions -> two partial sums per core; host combines.
"""

import os
import numpy as np
from contextlib import ExitStack

from concourse import bacc, bass, tile, mybir
from concourse.bass_isa import ReduceOp
from concourse.bass_utils import run_bass_kernel_spmd
from concourse.dve_spec import Spec, Src0, Src1, MaxNeg, maxx, lower
from concourse.dve_uop import AluInp, DveOpSpec
from concourse.dve_ops import (
    DveOp, OPS, has_src1, CUSTOM_DVE_SPECS,
    _SUB_OPCODE_FOR_NAME, _CUSTOM_DVE_ROW_BASE, _COMPILE_CACHE,
    get_dve_sub_opcode,
)

_B, _C, _H, _W = 8, 3, 64, 64
_P = 2048
_Q = _H * _W  # 4096
_K = 5
_NCORES = 8
_F32 = mybir.dt.float32
_NEG = -3.0e38

# Stashed BassKernelResults from the most recent kernel() call (for test.py).
LAST_RESULTS = None


class _HandEditedMaxMaxOp(DveOp):
    """Custom fused DVE op (uops ship inside the NEFF, no firmware dep):
        out[p,k]     = max(in0[p,k], in1[p,k])   -- elementwise accumulate
        accum_out[p] = max_k in0[p,k]            -- row reduce of in0 ONLY
    The native TENSOR_TENSOR_REDUCE opcode is not implemented by the stock
    DVE firmware on this runtime, and the Spec-level `accum=` folds the body
    output (which would contaminate the row max with in1's history), so the
    steady-state uop is hand-edited: the accum ALU's operand B is repointed
    from the body output (PREV_ALU_OUT) to the raw Src0 delay lane."""

    def compile(self, ver):
        key = (self.name, ver)
        if (r := _COMPILE_CACHE.get(key)) is not None:
            return r
        uops = lower(self.spec, ver=ver)
        assert len(uops) == 2
        uops[1].datapath_config[1].alu_src1 = AluInp.PREV_DELAY_0
        r = DveOpSpec(
            name=self.name,
            opcode=get_dve_sub_opcode(self.name),
            uops=uops,
            rd1_en=has_src1(self.spec),
        )
        _COMPILE_CACHE[key] = r
        return r


def _register_chamfer_op():
    name = "CHAMFER_MAX_SRC0MAX"
    if name in _SUB_OPCODE_FOR_NAME:
        return next(op for op in OPS if op.name == name)
    spec = Spec(
        body=maxx(Src0, Src1),
        accum=maxx,
        accum_init=MaxNeg,
        reference=lambda in0, in1, c0, c1, c2: (
            np.maximum(in0, in1),
            in0.max(axis=-1, keepdims=True),
        ),
    )
    _SUB_OPCODE_FOR_NAME[name] = _CUSTOM_DVE_ROW_BASE + len(OPS)
    op = _HandEditedMaxMaxOp(name, spec, subdim=False, uops_sha={})
    OPS.append(op)
    CUSTOM_DVE_SPECS[name] = spec
    return op


_CHAMFER_OP = _register_chamfer_op()


def _build_nc(repeat=1):
    nc = bacc.Bacc("TRN2", target_bir_lowering=False, debug=False)
    # a and b packed in one DRAM tensor; split into two DMAs so the first
    # half of the compute can start before the second half's data lands.
    # (The fused-LDW fp32 matmul struct can hold only ONE sync wait, so each
    # matmul must depend on at most one DMA.)
    ab_dram = nc.dram_tensor("ab", [_K, _P + _Q], _F32, kind="ExternalInput").ap()
    out_dram = nc.dram_tensor("out", [1, 2], _F32, kind="ExternalOutput").ap()

    n_ptile = _P // 128          # 16 stationary tiles
    fd = 2048                    # free-dim per PSUM supertile (4 banks)
    n_qsuper = _Q // fd          # 2
    n_mm = fd // 512             # 4 matmuls per supertile

    with ExitStack() as ctx:
        tc = ctx.enter_context(tile.TileContext(nc))
        sbuf = ctx.enter_context(tc.tile_pool(name="sbuf", bufs=1))
        psum = ctx.enter_context(tc.tile_pool(name="psum", bufs=2, space="PSUM"))

        ab_sb = sbuf.tile([_K, _P + _Q], _F32)
        # DMA 1: a (cols 0:P) + b first half; DMA 2: b second half.
        nc.sync.dma_start(ab_sb[:, 0:_P + fd], ab_dram[:, 0:_P + fd])
        nc.sync.dma_start(ab_sb[:, _P + fd:], ab_dram[:, _P + fd:])

        runq = sbuf.tile([128, _Q], _F32)
        nc.gpsimd.memset(runq[:], _NEG)
        rowmax = sbuf.tile([128, n_ptile * n_qsuper], _F32)
        qsum_h = sbuf.tile([128, n_qsuper], _F32)

        # qj OUTER: each runq half is final at the half-way point, so its
        # partition-reduce + row-sum epilogue hides under the other half's
        # main loop.  (`repeat` re-runs the whole body for scaling-based
        # timing measurements; results are unchanged.)
        for qj in [j for _ in range(repeat) for j in range(n_qsuper)]:
            for pi in range(n_ptile):
                lhsT = ab_sb[:, pi * 128:(pi + 1) * 128]
                pt = psum.tile([128, fd], _F32, tag="pt")
                for qk in range(n_mm):
                    q0 = _P + qj * fd + qk * 512
                    nc.tensor.matmul(
                        pt[:, qk * 512:(qk + 1) * 512],
                        lhsT,
                        ab_sb[:, q0:q0 + 512],
                        start=True,
                        stop=True,
                    )
                col = pi * n_qsuper + qj
                qs = runq[:, qj * fd:(qj + 1) * fd]
                nc.vector._custom_dve(
                    _CHAMFER_OP,
                    out=qs,
                    in0=pt[:],
                    in1=qs,
                    accum_out=rowmax[:, col:col + 1],
                )
            # per-half epilogue: max over partitions (gpsimd), then row-sum
            # on the otherwise-idle ACT engine (accum_out of an in-place
            # copy); every partition holds the same row, so row 0's sum is
            # sum_q max_p for this half.
            qs = runq[:, qj * fd:(qj + 1) * fd]
            nc.gpsimd.partition_all_reduce(qs, qs, 128, ReduceOp.max)
            nc.scalar.activation(
                qs, qs, mybir.ActivationFunctionType.Copy,
                accum_out=qsum_h[:, qj:qj + 1],
            )

        # sum_p (max_q D_neg): max over the n_qsuper columns per row, sum
        # over rows/partitions.
        rm2 = sbuf.tile([128, n_ptile], _F32)
        nc.vector.tensor_reduce(
            rm2[:],
            rowmax[:].rearrange("p (a b) -> p a b", b=n_qsuper),
            axis=mybir.AxisListType.X,
            op=mybir.AluOpType.max,
        )
        rsum = sbuf.tile([128, 1], _F32)
        nc.vector.tensor_reduce(
            rsum[:], rm2[:], axis=mybir.AxisListType.X, op=mybir.AluOpType.add
        )
        nc.gpsimd.partition_all_reduce(rsum[:], rsum[:], 128, ReduceOp.add)

        qsum = sbuf.tile([128, 1], _F32)
        nc.vector.tensor_reduce(
            qsum[:], qsum_h[:], axis=mybir.AxisListType.X, op=mybir.AluOpType.add
        )

        nc.sync.dma_start(out_dram[0:1, 0:1], rsum[0:1, 0:1])
        nc.sync.dma_start(out_dram[0:1, 1:2], qsum[0:1, 0:1])
    nc.compile()
    return nc


def _pack_inputs(y, t, weights):
    """Build per-core augmented factor matrices (numpy, O((p+q)*c) per core)."""
    w2 = (weights * weights).astype(np.float32)
    in_maps = []
    for i in range(_NCORES):
        yq = y[i].reshape(_C, _Q)
        ti = t[i]
        a = np.empty((_K, _P), np.float32)
        a[0:_C] = (2.0 * w2)[:, None] * ti.T
        a[_C] = -(w2[None, :] * ti * ti).sum(axis=1)
        a[_C + 1] = -1.0
        ab = np.empty((_K, _P + _Q), np.float32)
        ab[:, :_P] = a
        ab[0:_C, _P:] = yq
        ab[_C, _P:] = 1.0
        ab[_C + 1, _P:] = (w2[:, None] * yq * yq).sum(axis=0)
        in_maps.append({"ab": ab})
    return in_maps


def kernel(y, t, weights):
    global LAST_RESULTS
    y = np.asarray(y, dtype=np.float32)
    t = np.asarray(t, dtype=np.float32)
    weights = np.asarray(weights, dtype=np.float32)
    assert y.shape == (_B, _C, _H, _W) and t.shape == (_B, _P, _C)

    in_maps = _pack_inputs(y, t, weights)
    nc = _build_nc()
    trace = bool(os.environ.get("BASS_CHAMFER_TRACE"))
    res = run_bass_kernel_spmd(
        nc, in_maps, core_ids=list(range(_NCORES)), trace=trace
    )
    LAST_RESULTS = res

    bp_neg = sum(float(r["out"][0, 0]) for r in res.results)
    bq_neg = sum(float(r["out"][0, 1]) for r in res.results)
    total = -(bp_neg / (_B * _P) + bq_neg / (_B * _Q))
    return np.float32(total)


# revision 20
# speedup vs baseline: 64.6431x; 2.5720x over previous
"""Weighted Chamfer-MSE kernel for Trainium2 (8 NeuronCores, Bass/Tile).

Reference computes, per batch element:
    D[p, q]  = sum_c w[c]^2 * (t[p, c] - y[c, q])^2        (p=2048, q=4096)
    out      = mean_{b,p} min_q D + mean_{b,q} min_p D

Strategy (data-parallel over batch, 1 element per core):
  * Host packs the distance computation into ONE matmul with augmented
    contraction dim K=5:
        D_neg[p, q] = sum_k a[k, p] * b[k, q]
        a = [2*w2_c*t_pc (c=0..2), -wt2_p, -1],  b = [y_cq (c=0..2), 1, wy2_q]
    so D_neg = -D, and both min-reductions become max-reductions.
  * On device, PE produces D_neg in [128, 2048] PSUM tiles; a single fused
    vector.tensor_tensor_reduce per tile maintains
       runq   = elementwise max over p-tiles   (-> min over p, per q)
       rowmax = per-row max                    (-> min over q, per p)
    i.e. the whole 2048x4096 matrix crosses the DVE exactly once.
  * Epilogue: gpsimd partition_all_reduce(max) over runq partitions, small
    DVE add-reductions -> two partial sums per core; host combines.
"""

import os
import numpy as np
from contextlib import ExitStack

from concourse import bacc, bass, tile, mybir
from concourse.bass_isa import ReduceOp
from concourse.bass_utils import run_bass_kernel_spmd
from concourse.dve_spec import Spec, Src0, Src1, MaxNeg, maxx, lower
from concourse.dve_uop import AluInp, DveOpSpec
from concourse.dve_ops import (
    DveOp, OPS, has_src1, CUSTOM_DVE_SPECS,
    _SUB_OPCODE_FOR_NAME, _CUSTOM_DVE_ROW_BASE, _COMPILE_CACHE,
    get_dve_sub_opcode,
)

_B, _C, _H, _W = 8, 3, 64, 64
_P = 2048
_Q = _H * _W  # 4096
_K = 5
_NCORES = 8
_F32 = mybir.dt.float32
_NEG = -3.0e38

# Stashed BassKernelResults from the most recent kernel() call (for test.py).
LAST_RESULTS = None


class _HandEditedMaxMaxOp(DveOp):
    """Custom fused DVE op (uops ship inside the NEFF, no firmware dep):
        out[p,k]     = max(in0[p,k], in1[p,k])   -- elementwise accumulate
        accum_out[p] = max_k in0[p,k]            -- row reduce of in0 ONLY
    The native TENSOR_TENSOR_REDUCE opcode is not implemented by the stock
    DVE firmware on this runtime, and the Spec-level `accum=` folds the body
    output (which would contaminate the row max with in1's history), so the
    steady-state uop is hand-edited: the accum ALU's operand B is repointed
    from the body output (PREV_ALU_OUT) to the raw Src0 delay lane."""

    def compile(self, ver):
        key = (self.name, ver)
        if (r := _COMPILE_CACHE.get(key)) is not None:
            return r
        uops = lower(self.spec, ver=ver)
        assert len(uops) == 2
        uops[1].datapath_config[1].alu_src1 = AluInp.PREV_DELAY_0
        r = DveOpSpec(
            name=self.name,
            opcode=get_dve_sub_opcode(self.name),
            uops=uops,
            rd1_en=has_src1(self.spec),
        )
        _COMPILE_CACHE[key] = r
        return r


def _register_chamfer_op():
    name = "CHAMFER_MAX_SRC0MAX"
    if name in _SUB_OPCODE_FOR_NAME:
        return next(op for op in OPS if op.name == name)
    spec = Spec(
        body=maxx(Src0, Src1),
        accum=maxx,
        accum_init=MaxNeg,
        reference=lambda in0, in1, c0, c1, c2: (
            np.maximum(in0, in1),
            in0.max(axis=-1, keepdims=True),
        ),
    )
    _SUB_OPCODE_FOR_NAME[name] = _CUSTOM_DVE_ROW_BASE + len(OPS)
    op = _HandEditedMaxMaxOp(name, spec, subdim=False, uops_sha={})
    OPS.append(op)
    CUSTOM_DVE_SPECS[name] = spec
    return op


_CHAMFER_OP = _register_chamfer_op()


def _build_nc(repeat=1):
    nc = bacc.Bacc("TRN2", target_bir_lowering=False, debug=False)
    # a and b packed in one DRAM tensor; split into two DMAs so the first
    # half of the compute can start before the second half's data lands.
    # (The fused-LDW fp32 matmul struct can hold only ONE sync wait, so each
    # matmul must depend on at most one DMA.)
    ab_dram = nc.dram_tensor(
        "ab", [_K, _P + _Q], mybir.dt.float32r, kind="ExternalInput"
    ).ap()
    out_dram = nc.dram_tensor("out", [1, 2], _F32, kind="ExternalOutput").ap()

    n_ptile = _P // 128          # 16 stationary tiles
    fd = 2048                    # free-dim per PSUM supertile (4 banks)
    n_qsuper = _Q // fd          # 2
    n_mm = fd // 512             # 4 matmuls per supertile

    with ExitStack() as ctx:
        tc = ctx.enter_context(tile.TileContext(nc))
        sbuf = ctx.enter_context(tc.tile_pool(name="sbuf", bufs=1))
        psum = ctx.enter_context(tc.tile_pool(name="psum", bufs=2, space="PSUM"))

        ab_sb = sbuf.tile([_K, _P + _Q], mybir.dt.float32r)
        # DMA 1: a (cols 0:P) + b first half; DMA 2: b second half.
        nc.sync.dma_start(ab_sb[:, 0:_P + fd], ab_dram[:, 0:_P + fd])
        nc.sync.dma_start(ab_sb[:, _P + fd:], ab_dram[:, _P + fd:])

        runq = sbuf.tile([128, _Q], _F32)
        nc.gpsimd.memset(runq[:], _NEG)
        rowmax = sbuf.tile([128, n_ptile * n_qsuper], _F32)
        qsum_h = sbuf.tile([128, n_qsuper], _F32)

        # qj OUTER: each runq half is final at the half-way point, so its
        # partition-reduce + row-sum epilogue hides under the other half's
        # main loop.  (`repeat` re-runs the whole body for scaling-based
        # timing measurements; results are unchanged.)
        for qj in [j for _ in range(repeat) for j in range(n_qsuper)]:
            for pi in range(n_ptile):
                lhsT = ab_sb[:, pi * 128:(pi + 1) * 128]
                pt = psum.tile([128, fd], _F32, tag="pt")
                for qk in range(n_mm):
                    q0 = _P + qj * fd + qk * 512
                    # float32r: the PE's replicated-fp32 mode runs at full
                    # rate for moving dim >= 256 (plain float32 matmuls cost
                    # 4 cycles/row).
                    nc.tensor.matmul(
                        pt[:, qk * 512:(qk + 1) * 512],
                        lhsT,
                        ab_sb[:, q0:q0 + 512],
                        start=True,
                        stop=True,
                    )
                col = pi * n_qsuper + qj
                qs = runq[:, qj * fd:(qj + 1) * fd]
                nc.vector._custom_dve(
                    _CHAMFER_OP,
                    out=qs,
                    in0=pt[:],
                    in1=qs,
                    accum_out=rowmax[:, col:col + 1],
                )
            # per-half epilogue: max over partitions (gpsimd), then row-sum
            # on the otherwise-idle ACT engine (accum_out of an in-place
            # copy); every partition holds the same row, so row 0's sum is
            # sum_q max_p for this half.
            qs = runq[:, qj * fd:(qj + 1) * fd]
            nc.gpsimd.partition_all_reduce(qs, qs, 128, ReduceOp.max)
            nc.scalar.activation(
                qs, qs, mybir.ActivationFunctionType.Copy,
                accum_out=qsum_h[:, qj:qj + 1],
            )

        # sum_p (max_q D_neg): max over the n_qsuper columns per row, sum
        # over rows/partitions.
        rm2 = sbuf.tile([128, n_ptile], _F32)
        nc.vector.tensor_reduce(
            rm2[:],
            rowmax[:].rearrange("p (a b) -> p a b", b=n_qsuper),
            axis=mybir.AxisListType.X,
            op=mybir.AluOpType.max,
        )
        rsum = sbuf.tile([128, 1], _F32)
        nc.vector.tensor_reduce(
            rsum[:], rm2[:], axis=mybir.AxisListType.X, op=mybir.AluOpType.add
        )
        nc.gpsimd.partition_all_reduce(rsum[:], rsum[:], 128, ReduceOp.add)

        qsum = sbuf.tile([128, 1], _F32)
        nc.vector.tensor_reduce(
            qsum[:], qsum_h[:], axis=mybir.AxisListType.X, op=mybir.AluOpType.add
        )

        nc.sync.dma_start(out_dram[0:1, 0:1], rsum[0:1, 0:1])
        nc.sync.dma_start(out_dram[0:1, 1:2], qsum[0:1, 0:1])
    nc.compile()
    return nc


def _pack_inputs(y, t, weights):
    """Build per-core augmented factor matrices (numpy, O((p+q)*c) per core)."""
    w2 = (weights * weights).astype(np.float32)
    in_maps = []
    for i in range(_NCORES):
        yq = y[i].reshape(_C, _Q)
        ti = t[i]
        a = np.empty((_K, _P), np.float32)
        a[0:_C] = (2.0 * w2)[:, None] * ti.T
        a[_C] = -(w2[None, :] * ti * ti).sum(axis=1)
        a[_C + 1] = -1.0
        ab = np.empty((_K, _P + _Q), np.float32)
        ab[:, :_P] = a
        ab[0:_C, _P:] = yq
        ab[_C, _P:] = 1.0
        ab[_C + 1, _P:] = (w2[:, None] * yq * yq).sum(axis=0)
        in_maps.append({"ab": ab})
    return in_maps


def kernel(y, t, weights):
    global LAST_RESULTS
    y = np.asarray(y, dtype=np.float32)
    t = np.asarray(t, dtype=np.float32)
    weights = np.asarray(weights, dtype=np.float32)
    assert y.shape == (_B, _C, _H, _W) and t.shape == (_B, _P, _C)

    in_maps = _pack_inputs(y, t, weights)
    nc = _build_nc()
    trace = bool(os.environ.get("BASS_CHAMFER_TRACE"))
    res = run_bass_kernel_spmd(
        nc, in_maps, core_ids=list(range(_NCORES)), trace=trace
    )
    LAST_RESULTS = res

    bp_neg = sum(float(r["out"][0, 0]) for r in res.results)
    bq_neg = sum(float(r["out"][0, 1]) for r in res.results)
    total = -(bp_neg / (_B * _P) + bq_neg / (_B * _Q))
    return np.float32(total)


# revision 21
# speedup vs baseline: 72.6553x; 1.1239x over previous
"""Weighted Chamfer-MSE kernel for Trainium2 (8 NeuronCores, Bass/Tile).

Reference computes, per batch element:
    D[p, q]  = sum_c w[c]^2 * (t[p, c] - y[c, q])^2        (p=2048, q=4096)
    out      = mean_{b,p} min_q D + mean_{b,q} min_p D

Strategy (data-parallel over batch, 1 element per core):
  * Host packs the distance computation into ONE matmul with augmented
    contraction dim K=5:
        D_neg[p, q] = sum_k a[k, p] * b[k, q]
        a = [2*w2_c*t_pc (c=0..2), -wt2_p, -1],  b = [y_cq (c=0..2), 1, wy2_q]
    so D_neg = -D, and both min-reductions become max-reductions.
  * On device, PE produces D_neg in [128, 2048] PSUM tiles using float32r
    matmuls (full-rate fp32-replicated mode; plain fp32 is 4x slower); a
    single fused custom DVE op per tile maintains
       runq   = elementwise max over p-tiles   (-> min over p, per q)
       rowmax = per-row max of in0 only        (-> min over q, per p)
    i.e. the whole 2048x4096 matrix crosses the DVE exactly once.
  * Epilogue per q-half (overlapped with the other half's main loop):
    gpsimd partition_all_reduce(max) over runq partitions + row-sum on the
    ScalarE via activation accum_out; host combines the per-core scalars.
"""

import os
import numpy as np
from contextlib import ExitStack

from concourse import bacc, bass, tile, mybir
from concourse.bass_isa import ReduceOp
from concourse.bass_utils import run_bass_kernel_spmd
from concourse.dve_spec import Spec, Src0, Src1, MaxNeg, maxx, lower
from concourse.dve_uop import AluInp, DveOpSpec
from concourse.dve_ops import (
    DveOp, OPS, has_src1, CUSTOM_DVE_SPECS,
    _SUB_OPCODE_FOR_NAME, _CUSTOM_DVE_ROW_BASE, _COMPILE_CACHE,
    get_dve_sub_opcode,
)

_B, _C, _H, _W = 8, 3, 64, 64
_P = 2048
_Q = _H * _W  # 4096
_K = 5
_NCORES = 8
_F32 = mybir.dt.float32
_NEG = -3.0e38

# Stashed BassKernelResults from the most recent kernel() call (for test.py).
LAST_RESULTS = None


class _HandEditedMaxMaxOp(DveOp):
    """Custom fused DVE op (uops ship inside the NEFF, no firmware dep):
        out[p,k]     = max(in0[p,k], in1[p,k])   -- elementwise accumulate
        accum_out[p] = max_k in0[p,k]            -- row reduce of in0 ONLY
    The native TENSOR_TENSOR_REDUCE opcode is not implemented by the stock
    DVE firmware on this runtime, and the Spec-level `accum=` folds the body
    output (which would contaminate the row max with in1's history), so the
    steady-state uop is hand-edited: the accum ALU's operand B is repointed
    from the body output (PREV_ALU_OUT) to the raw Src0 delay lane."""

    def compile(self, ver):
        key = (self.name, ver)
        if (r := _COMPILE_CACHE.get(key)) is not None:
            return r
        uops = lower(self.spec, ver=ver)
        assert len(uops) == 2
        uops[1].datapath_config[1].alu_src1 = AluInp.PREV_DELAY_0
        r = DveOpSpec(
            name=self.name,
            opcode=get_dve_sub_opcode(self.name),
            uops=uops,
            rd1_en=has_src1(self.spec),
        )
        _COMPILE_CACHE[key] = r
        return r


def _register_chamfer_op():
    name = "CHAMFER_MAX_SRC0MAX"
    if name in _SUB_OPCODE_FOR_NAME:
        return next(op for op in OPS if op.name == name)
    spec = Spec(
        body=maxx(Src0, Src1),
        accum=maxx,
        accum_init=MaxNeg,
        reference=lambda in0, in1, c0, c1, c2: (
            np.maximum(in0, in1),
            in0.max(axis=-1, keepdims=True),
        ),
    )
    _SUB_OPCODE_FOR_NAME[name] = _CUSTOM_DVE_ROW_BASE + len(OPS)
    op = _HandEditedMaxMaxOp(name, spec, subdim=False, uops_sha={})
    OPS.append(op)
    CUSTOM_DVE_SPECS[name] = spec
    return op


_CHAMFER_OP = _register_chamfer_op()


def _build_nc(repeat=1):
    nc = bacc.Bacc("TRN2", target_bir_lowering=False, debug=False)
    # a and b packed in one DRAM tensor; split into two DMAs so the first
    # half of the compute can start before the second half's data lands.
    # (The fused-LDW fp32 matmul struct can hold only ONE sync wait, so each
    # matmul must depend on at most one DMA.)
    ab_dram = nc.dram_tensor(
        "ab", [_K, _P + _Q], mybir.dt.float32r, kind="ExternalInput"
    ).ap()
    out_dram = nc.dram_tensor("out", [1, 2], _F32, kind="ExternalOutput").ap()

    n_ptile = _P // 128          # 16 stationary tiles
    fd = 2048                    # free-dim per PSUM supertile (4 banks)
    n_qsuper = _Q // fd          # 2
    n_mm = fd // 512             # 4 matmuls per supertile

    with ExitStack() as ctx:
        tc = ctx.enter_context(tile.TileContext(nc))
        sbuf = ctx.enter_context(tc.tile_pool(name="sbuf", bufs=1))
        psum = ctx.enter_context(tc.tile_pool(name="psum", bufs=2, space="PSUM"))

        ab_sb = sbuf.tile([_K, _P + _Q], mybir.dt.float32r)
        # DMA 1: a (cols 0:P) + b first half; DMA 2: b second half.
        nc.sync.dma_start(ab_sb[:, 0:_P + fd], ab_dram[:, 0:_P + fd])
        nc.sync.dma_start(ab_sb[:, _P + fd:], ab_dram[:, _P + fd:])

        runq = sbuf.tile([128, _Q], _F32)
        nc.gpsimd.memset(runq[:], _NEG)
        rowmax = sbuf.tile([128, n_ptile * n_qsuper], _F32)
        qsum_h = sbuf.tile([128, n_qsuper], _F32)

        # qj OUTER: each runq half is final at the half-way point, so its
        # partition-reduce + row-sum epilogue hides under the other half's
        # main loop.  (`repeat` re-runs the whole body for scaling-based
        # timing measurements; results are unchanged.)
        for qj in [j for _ in range(repeat) for j in range(n_qsuper)]:
            for pi in range(n_ptile):
                lhsT = ab_sb[:, pi * 128:(pi + 1) * 128]
                pt = psum.tile([128, fd], _F32, tag="pt")
                for qk in range(n_mm):
                    q0 = _P + qj * fd + qk * 512
                    # float32r: the PE's replicated-fp32 mode runs at full
                    # rate for moving dim >= 256 (plain float32 matmuls cost
                    # 4 cycles/row).
                    nc.tensor.matmul(
                        pt[:, qk * 512:(qk + 1) * 512],
                        lhsT,
                        ab_sb[:, q0:q0 + 512],
                        start=True,
                        stop=True,
                    )
                col = pi * n_qsuper + qj
                qs = runq[:, qj * fd:(qj + 1) * fd]
                nc.vector._custom_dve(
                    _CHAMFER_OP,
                    out=qs,
                    in0=pt[:],
                    in1=qs,
                    accum_out=rowmax[:, col:col + 1],
                )
            # per-half epilogue: max over partitions (gpsimd), then row-sum
            # on the otherwise-idle ACT engine (accum_out of an in-place
            # copy); every partition holds the same row, so row 0's sum is
            # sum_q max_p for this half.
            qs = runq[:, qj * fd:(qj + 1) * fd]
            nc.gpsimd.partition_all_reduce(qs, qs, 128, ReduceOp.max)
            nc.scalar.activation(
                qs, qs, mybir.ActivationFunctionType.Copy,
                accum_out=qsum_h[:, qj:qj + 1],
            )

        # sum_p (max_q D_neg): max over the n_qsuper columns per row, sum
        # over rows/partitions.
        rm2 = sbuf.tile([128, n_ptile], _F32)
        nc.vector.tensor_reduce(
            rm2[:],
            rowmax[:].rearrange("p (a b) -> p a b", b=n_qsuper),
            axis=mybir.AxisListType.X,
            op=mybir.AluOpType.max,
        )
        rsum = sbuf.tile([128, 1], _F32)
        nc.vector.tensor_reduce(
            rsum[:], rm2[:], axis=mybir.AxisListType.X, op=mybir.AluOpType.add
        )
        nc.gpsimd.partition_all_reduce(rsum[:], rsum[:], 128, ReduceOp.add)

        qsum = sbuf.tile([128, 1], _F32)
        nc.vector.tensor_reduce(
            qsum[:], qsum_h[:], axis=mybir.AxisListType.X, op=mybir.AluOpType.add
        )

        nc.sync.dma_start(out_dram[0:1, 0:1], rsum[0:1, 0:1])
        nc.sync.dma_start(out_dram[0:1, 1:2], qsum[0:1, 0:1])
    nc.compile()
    return nc


def _pack_inputs(y, t, weights):
    """Build per-core augmented factor matrices (numpy, O((p+q)*c) per core)."""
    w2 = (weights * weights).astype(np.float32)
    in_maps = []
    for i in range(_NCORES):
        yq = y[i].reshape(_C, _Q)
        ti = t[i]
        a = np.empty((_K, _P), np.float32)
        a[0:_C] = (2.0 * w2)[:, None] * ti.T
        a[_C] = -(w2[None, :] * ti * ti).sum(axis=1)
        a[_C + 1] = -1.0
        ab = np.empty((_K, _P + _Q), np.float32)
        ab[:, :_P] = a
        ab[0:_C, _P:] = yq
        ab[_C, _P:] = 1.0
        ab[_C + 1, _P:] = (w2[:, None] * yq * yq).sum(axis=0)
        in_maps.append({"ab": ab})
    return in_maps


def kernel(y, t, weights):
    global LAST_RESULTS
    y = np.asarray(y, dtype=np.float32)
    t = np.asarray(t, dtype=np.float32)
    weights = np.asarray(weights, dtype=np.float32)
    assert y.shape == (_B, _C, _H, _W) and t.shape == (_B, _P, _C)

    in_maps = _pack_inputs(y, t, weights)
    nc = _build_nc()
    trace = bool(os.environ.get("BASS_CHAMFER_TRACE"))
    res = run_bass_kernel_spmd(
        nc, in_maps, core_ids=list(range(_NCORES)), trace=trace
    )
    LAST_RESULTS = res

    bp_neg = sum(float(r["out"][0, 0]) for r in res.results)
    bq_neg = sum(float(r["out"][0, 1]) for r in res.results)
    total = -(bp_neg / (_B * _P) + bq_neg / (_B * _Q))
    return np.float32(total)
